# revision 1
# baseline (speedup 1.0000x reference)
"""Trainium2 Bass kernel for CommunicativeMessagePassing (D-MPNN bond-message GNN).

Self-contained: takes full inputs, shards across 8 NeuronCores, returns full output.

Math (dead code removed -- the reference's H_a / a_max / gate chain never reaches
the output):
    H_b = relu(concat(V[v], E_feat) @ Wi_bond.T)
    2x:  a_sum = segment_sum(H_b, w); H_b += relu((H_b + (a_sum[v] - H_b[rev]) @ Wh.T) @ Wf.T)
    a_sum = segment_sum(H_b, w); out = relu(concat(V, a_sum) @ Wo_atom.T)

Per-edge update is rewritten as relu(Wf.h + A4[src] - C.h_pair) with C = Wf@Wh and
A4 = C . a_sum (per atom), so the only cross-core data is an AllGather of A4.

Sharding: atoms split into 8 contiguous ranges. Each core stores bond states for
edges with local destination (L, round-major degree-sorted order for dense
segment-sum) plus their reverse edges (R, pair-aligned), updating both. R rows'
A4 term is a dense slice of the local A4; L rows' A4[src] is an indirect row
gather from the allgathered table.
"""
import sys
sys.path.insert(0, "/opt/trn_rl_repo")
import numpy as np
import ml_dtypes
BF = ml_dtypes.bfloat16

NCORES = 8
P = 128
CHUNK = 512  # edge columns per phase-B tile / matmul free dim

_cache = {}


# ---------------------------------------------------------------- host preprocessing
def _preprocess(V, E_feat, edge_index, rev_edge_index):
    N, DV = V.shape
    E, DE = E_feat.shape
    v = np.asarray(edge_index[0], np.int64)
    w = np.asarray(edge_index[1], np.int64)
    rev = np.asarray(rev_edge_index, np.int64)
    ASH = N // NCORES
    APAD = ((ASH + P - 1) // P) * P

    shard = w // ASH
    src_shard = v // ASH

    per = []
    for c in range(NCORES):
        eids = np.nonzero(shard == c)[0]
        wl = w[eids] - c * ASH
        deg = np.bincount(wl, minlength=ASH)
        order = np.argsort(-deg, kind="stable")      # rank -> atom (local)
        rank_of = np.empty(ASH, np.int64)
        rank_of[order] = np.arange(ASH)
        ar = rank_of[wl]
        o2 = np.lexsort((eids, ar))
        eids_s, ar_s = eids[o2], ar[o2]
        # position of each edge within its atom's run
        if len(ar_s):
            runs = np.r_[0, np.nonzero(np.diff(ar_s))[0] + 1]
            lens = np.diff(np.r_[runs, len(ar_s)])
            pos = np.arange(len(ar_s)) - np.repeat(runs, lens)
        else:
            pos = np.zeros(0, np.int64)
        per.append(dict(eids=eids_s, ar=ar_s, pos=pos, deg=deg, order=order, rank_of=rank_of))

    maxdeg = max(int(p["deg"].max()) for p in per) if E else 0
    n_r = np.zeros(maxdeg, np.int64)
    for p in per:
        cnt = np.bincount(p["deg"], minlength=maxdeg + 1)
        gt = ASH - np.cumsum(cnt)[:maxdeg]          # atoms with deg > r
        n_r = np.maximum(n_r, gt)
    starts = np.r_[0, np.cumsum(n_r)]
    K = int(starts[-1])
    KP = ((K + CHUNK - 1) // CHUNK) * CHUNK

    # global A4-table row of an atom a: shard(a)*APAD + rank within its shard
    def a4row(atoms):
        s = atoms // ASH
        r = np.empty(len(atoms), np.int64)
        for c in range(NCORES):
            m = s == c
            if m.any():
                r[m] = per[c]["rank_of"][atoms[m] - c * ASH]
        return s * APAD + r

    cores = []
    for c in range(NCORES):
        p = per[c]
        cols = starts[p["pos"]] + p["ar"]
        L_eid = np.full(KP, -1, np.int64)
        L_eid[cols] = p["eids"]
        mask = L_eid >= 0
        R_eid = np.full(KP, -1, np.int64)
        R_eid[mask] = rev[L_eid[mask]]

        G = np.full(KP, ASH, np.int64)              # pad -> zero row of shard 0
        G[mask] = a4row(v[L_eid[mask]])
        gidx = G.reshape(KP // P, P).T.astype(np.int32).copy()   # [128, KP/128]

        # X staging, feature-major [DV+DE, 2*KP]
        X = np.zeros((DV + DE, 2 * KP), BF)
        le = L_eid[mask]
        re_ = R_eid[mask]
        X[:DV, :KP][:, mask] = V[v[le]].T.astype(BF)
        X[DV:, :KP][:, mask] = E_feat[le].T.astype(BF)
        X[:DV, KP:][:, mask] = V[v[re_]].T.astype(BF)
        X[DV:, KP:][:, mask] = E_feat[re_].T.astype(BF)

        Vfm = np.zeros((DV, APAD), np.float32)
        Vfm[:, :ASH] = V[c * ASH + p["order"]].T
        maskrep = np.broadcast_to(mask.astype(np.float32)[None, :], (P, KP)).copy()
        cores.append(dict(gidx=gidx, X=X, Vfm=Vfm, order=p["order"],
                          L_eid=L_eid, R_eid=R_eid, mask=mask, maskrep=maskrep))

    return dict(N=N, E=E, DV=DV, DE=DE, ASH=ASH, APAD=APAD, KP=KP,
                starts=starts.astype(np.int64), n_r=n_r, cores=cores)


def _weights(Wi_bond, Wh_bond, Wf_bond, Wo_atom, DV):
    C = (Wf_bond @ Wh_bond).astype(np.float32)
    return dict(
        WiT=np.ascontiguousarray(Wi_bond.T.astype(BF)),       # [DV+DE, DH]
        WfT=np.ascontiguousarray(Wf_bond.T.astype(BF)),       # [DH, DH]
        CT=np.ascontiguousarray(C.T.astype(np.float32)),
        CnT=np.ascontiguousarray((-C.T).astype(BF)),
        WoTv=np.ascontiguousarray(Wo_atom.T[:DV].astype(np.float32)),  # [DV, DH]
        WoTs=np.ascontiguousarray(Wo_atom.T[DV:].astype(np.float32)),  # [DH, DH]
        ident=np.eye(P, dtype=np.float32),
        identb=np.eye(P, dtype=BF),
    )


# ---------------------------------------------------------------- bass program
def _build(meta, DH=256, DEPTH_ITERS=2):
    import concourse.bass as bass
    import concourse.bacc as bacc
    import concourse.tile as tile
    from concourse import mybir

    F32R = mybir.dt.float32r
    F32 = mybir.dt.float32
    BF16 = mybir.dt.bfloat16
    KP, APAD, DV, DE = meta["KP"], meta["APAD"], meta["DV"], meta["DE"]
    starts, n_r = meta["starts"], meta["n_r"]
    R_ROUNDS = len(n_r)
    DXT = DV + DE
    NFH = DH // P                      # feature halves (2)
    NA_CH = (APAD + CHUNK - 1) // CHUNK
    NT = KP // CHUNK                   # phase-B tiles per half

    nc = bacc.Bacc("TRN2", target_bir_lowering=False, debug=False, num_devices=NCORES)

    x_in = nc.dram_tensor("x", [DXT, 2 * KP], BF16, kind="ExternalInput")
    gidx_in = nc.dram_tensor("gidx", [P, KP // P], mybir.dt.int32, kind="ExternalInput")
    vfm_in = nc.dram_tensor("vfm", [DV, APAD], F32R, kind="ExternalInput")
    mask_in = nc.dram_tensor("mask", [P, KP], F32, kind="ExternalInput")
    wiT_in = nc.dram_tensor("wiT", [DXT, DH], BF16, kind="ExternalInput")
    wfT_in = nc.dram_tensor("wfT", [DH, DH], BF16, kind="ExternalInput")
    ct_in = nc.dram_tensor("cT", [DH, DH], F32R, kind="ExternalInput")
    cnT_in = nc.dram_tensor("cnT", [DH, DH], BF16, kind="ExternalInput")
    woTv_in = nc.dram_tensor("woTv", [DV, DH], F32R, kind="ExternalInput")
    woTs_in = nc.dram_tensor("woTs", [DH, DH], F32R, kind="ExternalInput")
    id_in = nc.dram_tensor("ident", [P, P], F32R, kind="ExternalInput")
    idb_in = nc.dram_tensor("identb", [P, P], BF16, kind="ExternalInput")
    zero_in = nc.dram_tensor("zeros", [P, CHUNK], F32R, kind="ExternalInput")
    out_ext = nc.dram_tensor("out", [DH, APAD], F32, kind="ExternalOutput")

    H = [nc.dram_tensor(f"hbuf{i}", [DH, 2 * KP], BF16) for i in range(2)]
    ccin = [nc.dram_tensor(f"ccin{i}", [APAD, DH], BF16) for i in range(DEPTH_ITERS)]
    ccout = [nc.dram_tensor(f"ccout{i}", [NCORES * APAD, DH], BF16, addr_space="Shared")
             for i in range(DEPTH_ITERS)]

    with tile.TileContext(nc) as tc:
        with (
            tc.tile_pool(name="wpool", bufs=1) as wp,
            tc.tile_pool(name="state", bufs=1) as st,
            tc.tile_pool(name="hin", bufs=3) as hp,
            tc.tile_pool(name="rout", bufs=3) as rp,
            tc.tile_pool(name="gpool", bufs=3) as gp,
            tc.tile_pool(name="psum", bufs=3, space="PSUM") as ps,
        ):
            # ---- resident weights
            def wload(src, rows, cols, dt=F32R):
                t = wp.tile([rows, cols], dt, tag=f"w{src.name}{rows}", name=f"w{src.name}{rows}")
                nc.sync.dma_start(out=t[:], in_=src[:rows, :cols])
                return t

            wiT0 = wp.tile([P, DH], BF16, name="wiT0")
            nc.sync.dma_start(out=wiT0[:], in_=wiT_in[:P, :])
            wiT1 = wp.tile([DXT - P, DH], BF16, name="wiT1")
            nc.sync.dma_start(out=wiT1[:], in_=wiT_in[P:DXT, :])
            wfT = [wp.tile([P, DH], BF16, tag=f"wf{k}", name=f"wf{k}") for k in range(NFH)]
            cT = [wp.tile([P, DH], F32R, tag=f"ct{k}", name=f"ct{k}") for k in range(NFH)]
            cnT = [wp.tile([P, DH], BF16, tag=f"cn{k}", name=f"cn{k}") for k in range(NFH)]
            for k in range(NFH):
                nc.sync.dma_start(out=wfT[k][:], in_=wfT_in[k * P:(k + 1) * P, :])
                nc.sync.dma_start(out=cT[k][:], in_=ct_in[k * P:(k + 1) * P, :])
                nc.sync.dma_start(out=cnT[k][:], in_=cnT_in[k * P:(k + 1) * P, :])
            woTv0 = wload(woTv_in, P, DH)
            woTv1 = wp.tile([DV - P, DH], F32, name="woTv1")
            nc.sync.dma_start(out=woTv1[:], in_=woTv_in[P:DV, :].bitcast(F32))
            woTs = [wp.tile([P, DH], F32R, tag=f"wo{k}", name=f"wo{k}") for k in range(NFH)]
            for k in range(NFH):
                nc.sync.dma_start(out=woTs[k][:], in_=woTs_in[k * P:(k + 1) * P, :])
            ident = wp.tile([P, P], F32R, name="identt")
            nc.sync.dma_start(out=ident[:], in_=id_in[:, :])
            identb = wp.tile([P, P], BF16, name="identb")
            nc.sync.dma_start(out=identb[:], in_=idb_in[:, :])
            gidx = wp.tile([P, KP // P], mybir.dt.int32, name="gidxt")
            nc.sync.dma_start(out=gidx[:], in_=gidx_in[:, :])

            # ---- persistent atom-state accumulators (feature-major)
            asum = [st.tile([P, APAD], F32R, tag=f"as{f}", name=f"as{f}") for f in range(NFH)]
            a4fm = asum  # A4 = C.asum computed in place (asum not needed once A4 is)

            # ---- init: H0 = relu(WiT.T @ X)
            for t in range(2 * KP // CHUNK):
                c0 = t * CHUNK
                x0 = hp.tile([P, CHUNK], BF16, tag="x0", name="x0")
                x1 = hp.tile([DXT - P, CHUNK], BF16, tag="x1", name="x1")
                nc.sync.dma_start(out=x0[:], in_=x_in[:P, c0:c0 + CHUNK])
                nc.sync.dma_start(out=x1[:], in_=x_in[P:DXT, c0:c0 + CHUNK])
                acc = ps.tile([P, NFH * CHUNK], F32, space="PSUM", tag="acc", name="acc")
                for f in range(NFH):
                    o = f * CHUNK
                    nc.tensor.matmul(out=acc[:, o:o + CHUNK], lhsT=wiT0[:, f * P:(f + 1) * P],
                                     rhs=x0[:], start=True, stop=False)
                    nc.tensor.matmul(out=acc[:, o:o + CHUNK],
                                     lhsT=wiT1[:, f * P:(f + 1) * P], rhs=x1[:],
                                     start=False, stop=True)
                for f in range(NFH):
                    h0 = rp.tile([P, CHUNK], BF16, tag=f"r{f}", name=f"r{f}")
                    nc.scalar.activation(out=h0[:], in_=acc[:, f * CHUNK:(f + 1) * CHUNK],
                                         func=mybir.ActivationFunctionType.Relu)
                    nc.sync.dma_start(out=H[0][f * P:(f + 1) * P, c0:c0 + CHUNK], in_=h0[:])

            # ---- helper: segment-sum reduction of H_L into asum (round-major slices)
            def reduction(hsrc):
                # zero tail columns never written by round 0
                n1 = int(n_r[0]) if R_ROUNDS else 0
                for f in range(NFH):
                    z = n1
                    while z < APAD:
                        zn = min(CHUNK, APAD - z)
                        nc.sync.dma_start(out=asum[f][:, z:z + zn], in_=zero_in[:, :zn])
                        z += zn
                bounds = list(starts)
                for t in range(NT):
                    c0 = t * CHUNK
                    c1 = min(c0 + CHUNK, KP)
                    if c0 >= starts[-1]:
                        break
                    hl = [hp.tile([P, CHUNK], BF16, tag=f"hl{f}", name=f"hl{f}") for f in range(NFH)]
                    for f in range(NFH):
                        nc.sync.dma_start(out=hl[f][:], in_=hsrc[f * P:(f + 1) * P, c0:c0 + CHUNK])
                    # split [c0, c1) by round boundaries
                    for r in range(R_ROUNDS):
                        a = max(c0, int(bounds[r]))
                        b = min(c1, int(bounds[r + 1]))
                        if a >= b:
                            continue
                        d0 = a - int(bounds[r])
                        for f in range(NFH):
                            if r == 0:
                                nc.vector.tensor_copy(out=asum[f][:, d0:d0 + (b - a)],
                                                      in_=hl[f][:, a - c0:b - c0])
                            else:
                                nc.vector.tensor_add(out=asum[f][:, d0:d0 + (b - a)],
                                                     in0=asum[f][:, d0:d0 + (b - a)],
                                                     in1=hl[f][:, a - c0:b - c0])

            # ---- helper: A4 = C.asum (fm), transpose to atom-major, DMA to cc input
            def a4_compute(it):
                for t in range(NA_CH):
                    c0 = t * CHUNK
                    c1 = min(c0 + CHUNK, APAD)
                    n = c1 - c0
                    acc = ps.tile([P, NFH * CHUNK], F32, space="PSUM", tag="acc", name="acc")
                    for f in range(NFH):
                        o = f * CHUNK
                        for k in range(NFH):
                            nc.tensor.matmul(out=acc[:, o:o + n],
                                             lhsT=cT[k][:, f * P:(f + 1) * P],
                                             rhs=asum[k][:, c0:c1],
                                             start=(k == 0), stop=(k == NFH - 1))
                    for f in range(NFH):
                        nc.vector.tensor_copy(out=a4fm[f][:, c0:c1], in_=acc[:, f * CHUNK:f * CHUNK + n])
                # transpose APAD columns of a4fm into [APAD, DH] rows
                for blk in range(APAD // P):
                    a0 = blk * P
                    tp = ps.tile([P, NFH * P], F32, space="PSUM", tag="tp", name="tp", bufs=2)
                    for f in range(NFH):
                        nc.tensor.matmul(out=tp[:, f * P:(f + 1) * P],
                                         lhsT=a4fm[f][:, a0:a0 + P], rhs=ident[:],
                                         start=True, stop=True)
                    stg = gp.tile([P, NFH * P], BF16, tag="tstg", name="tstg")
                    nc.vector.tensor_copy(out=stg[:], in_=tp[:])
                    nc.sync.dma_start(out=ccin[it][a0:a0 + P, :], in_=stg[:])

            # ---- iterations
            for it in range(DEPTH_ITERS):
                hsrc, hdst = H[it % 2], H[(it + 1) % 2]
                reduction(hsrc)
                a4_compute(it)
                nc.gpsimd.collective_compute(
                    "AllGather", mybir.AluOpType.bypass,
                    replica_groups=[list(range(NCORES))],
                    ins=[ccin[it].ap().opt()], outs=[ccout[it].ap().opt()],
                )
                # phase B: update all edge columns
                for t in range(NT):
                    c0 = t * CHUNK
                    hl = [hp.tile([P, CHUNK], BF16, tag=f"hl{f}", name=f"hl{f}") for f in range(NFH)]
                    hr = [hp.tile([P, CHUNK], BF16, tag=f"hr{f}", name=f"hr{f}") for f in range(NFH)]
                    for f in range(NFH):
                        nc.sync.dma_start(out=hl[f][:], in_=hsrc[f * P:(f + 1) * P, c0:c0 + CHUNK])
                        nc.sync.dma_start(out=hr[f][:], in_=hsrc[f * P:(f + 1) * P, KP + c0:KP + c0 + CHUNK])
                    # gather A4 rows for L sources: 4 groups of 128
                    g_t = gp.tile([P, (CHUNK // P) * DH], BF16, tag="gg", name="g_t")
                    for s in range(CHUNK // P):
                        grp = t * (CHUNK // P) + s
                        nc.gpsimd.indirect_dma_start(
                            out=g_t[:, s * DH:(s + 1) * DH], out_offset=None,
                            in_=ccout[it][:, :],
                            in_offset=bass.IndirectOffsetOnAxis(ap=gidx[:, grp:grp + 1], axis=0),
                        )
                    accL = ps.tile([P, NFH * CHUNK], F32, space="PSUM", tag="acc", name="acc")
                    accR = ps.tile([P, NFH * CHUNK], F32, space="PSUM", tag="acc", name="acc")
                    for f in range(NFH):
                        o = f * CHUNK
                        for k in range(NFH):
                            nc.tensor.matmul(out=accL[:, o:o + CHUNK], lhsT=wfT[k][:, f * P:(f + 1) * P],
                                             rhs=hl[k][:], start=(k == 0), stop=False)
                        for k in range(NFH):
                            nc.tensor.matmul(out=accL[:, o:o + CHUNK], lhsT=cnT[k][:, f * P:(f + 1) * P],
                                             rhs=hr[k][:], start=False, stop=False)
                        for s in range(CHUNK // P):
                            nc.tensor.matmul(out=accL[:, o + s * P:o + (s + 1) * P],
                                             lhsT=g_t[:, s * DH + f * P:s * DH + (f + 1) * P],
                                             rhs=identb[:], start=False, stop=(s == CHUNK // P - 1))
                        for k in range(NFH):
                            nc.tensor.matmul(out=accR[:, o:o + CHUNK], lhsT=wfT[k][:, f * P:(f + 1) * P],
                                             rhs=hr[k][:], start=(k == 0), stop=False)
                        for k in range(NFH):
                            nc.tensor.matmul(out=accR[:, o:o + CHUNK], lhsT=cnT[k][:, f * P:(f + 1) * P],
                                             rhs=hl[k][:], start=False, stop=(k == NFH - 1))
                    # R half: add dense local A4 (per round segment)
                    mt = gp.tile([P, CHUNK], F32, tag="mask", name="mask")
                    nc.sync.dma_start(out=mt[:], in_=mask_in[:, c0:c0 + CHUNK])
                    c1 = c0 + CHUNK
                    for r in range(R_ROUNDS):
                        a = max(c0, int(starts[r]))
                        b = min(c1, int(starts[r + 1]))
                        if a >= b:
                            continue
                        d0 = a - int(starts[r])
                        for f in range(NFH):
                            o = f * CHUNK
                            nc.vector.tensor_add(out=accR[:, o + a - c0:o + b - c0],
                                                 in0=accR[:, o + a - c0:o + b - c0],
                                                 in1=a4fm[f][:, d0:d0 + (b - a)].bitcast(F32))
                    # mask R-half arg (zero pad columns), then relu, residual, store
                    for f in range(NFH):
                        o = f * CHUNK
                        nc.vector.tensor_mul(out=accR[:, o:o + CHUNK],
                                             in0=accR[:, o:o + CHUNK], in1=mt[:])
                    for f in range(NFH):
                        rl = rp.tile([P, CHUNK], BF16, tag=f"r{f}", name=f"r{f}")
                        nc.scalar.activation(out=rl[:], in_=accL[:, f * CHUNK:(f + 1) * CHUNK],
                                             func=mybir.ActivationFunctionType.Relu)
                        nc.vector.tensor_add(out=rl[:], in0=rl[:], in1=hl[f][:])
                        nc.sync.dma_start(out=hdst[f * P:(f + 1) * P, c0:c0 + CHUNK], in_=rl[:])
                    for f in range(NFH):
                        rr = rp.tile([P, CHUNK], BF16, tag=f"rr{f}", name=f"rr{f}")
                        nc.scalar.activation(out=rr[:], in_=accR[:, f * CHUNK:(f + 1) * CHUNK],
                                             func=mybir.ActivationFunctionType.Relu)
                        nc.vector.tensor_add(out=rr[:], in0=rr[:], in1=hr[f][:])
                        nc.sync.dma_start(out=hdst[f * P:(f + 1) * P, KP + c0:KP + c0 + CHUNK], in_=rr[:])

            # ---- final: a_sum from last H, out = relu(WoT.T @ [V; a_sum])
            reduction(H[DEPTH_ITERS % 2])
            for t in range(NA_CH):
                c0 = t * CHUNK
                c1 = min(c0 + CHUNK, APAD)
                n = c1 - c0
                vt0 = hp.tile([P, CHUNK], F32R, tag="x0", name="x0")
                vt1 = hp.tile([DV - P, CHUNK], F32, tag="x1", name="xv1")
                nc.sync.dma_start(out=vt0[:, :n], in_=vfm_in[:P, c0:c1])
                nc.sync.dma_start(out=vt1[:, :n], in_=vfm_in[P:DV, c0:c1].bitcast(F32))
                acc = ps.tile([P, NFH * CHUNK], F32, space="PSUM", tag="acc", name="acc")
                for f in range(NFH):
                    o = f * CHUNK
                    nc.tensor.matmul(out=acc[:, o:o + n], lhsT=woTv0[:, f * P:(f + 1) * P],
                                     rhs=vt0[:, :n], start=True, stop=False)
                    nc.tensor.matmul(out=acc[:, o:o + n], lhsT=woTv1[:, f * P:(f + 1) * P],
                                     rhs=vt1[:, :n], start=False, stop=False)
                    for k in range(NFH):
                        nc.tensor.matmul(out=acc[:, o:o + n], lhsT=woTs[k][:, f * P:(f + 1) * P],
                                         rhs=asum[k][:, c0:c1], start=False, stop=(k == NFH - 1))
                for f in range(NFH):
                    ot = rp.tile([P, CHUNK], F32, tag=f"r{f}", name=f"o{f}")
                    nc.scalar.activation(out=ot[:, :n], in_=acc[:, f * CHUNK:f * CHUNK + n],
                                         func=mybir.ActivationFunctionType.Relu)
                    nc.sync.dma_start(out=out_ext[f * P:(f + 1) * P, c0:c1], in_=ot[:, :n])

    nc.compile()
    return nc


# ---------------------------------------------------------------- entry point
def _run(inputs, trace=False):
    from concourse.bass_utils import run_bass_kernel_spmd

    V = np.asarray(inputs["V"], np.float32)
    E_feat = np.asarray(inputs["E_feat"], np.float32)
    meta = _preprocess(V, E_feat, np.asarray(inputs["edge_index"]),
                       np.asarray(inputs["rev_edge_index"]))
    wts = _weights(np.asarray(inputs["Wi_bond"], np.float32),
                   np.asarray(inputs["Wh_bond"], np.float32),
                   np.asarray(inputs["Wf_bond"], np.float32),
                   np.asarray(inputs["Wo_atom"], np.float32), meta["DV"])

    key = (meta["KP"], meta["APAD"], tuple(meta["starts"].tolist()))
    if key not in _cache:
        _cache[key] = _build(meta)
    nc = _cache[key]

    in_maps = []
    for c in range(NCORES):
        cd = meta["cores"][c]
        in_maps.append({
            "x": cd["X"], "gidx": cd["gidx"], "vfm": cd["Vfm"], "mask": cd["maskrep"],
            "zeros": np.zeros((P, CHUNK), np.float32),
            "wiT": wts["WiT"], "wfT": wts["WfT"], "cT": wts["CT"], "cnT": wts["CnT"],
            "woTv": wts["WoTv"], "woTs": wts["WoTs"], "ident": wts["ident"],
            "identb": wts["identb"],
        })
    res = run_bass_kernel_spmd(nc, in_maps, core_ids=list(range(NCORES)), trace=trace)

    N, ASH, DH = meta["N"], meta["ASH"], 256
    out = np.empty((N, DH), np.float32)
    for c in range(NCORES):
        o = res.results[c]["out"]                    # [DH, APAD]
        order = meta["cores"][c]["order"]
        out[c * ASH + order] = o[:, :ASH].T
    return out, res.exec_time_ns


def kernel(**inputs) -> np.ndarray:
    out, _ = _run(inputs, trace=False)
    return out

